# revision 14
# baseline (speedup 1.0000x reference)
"""Multi-head attention (B=2, S=2048, D=1024, H=16) on 8 trn2 NeuronCores.

Sharding: core c handles heads {2c, 2c+1} for BOTH batches (tensor parallel by
head). Token axis is flattened b-major: T = B*S = 4096.
 - Q/K/V projections computed per-core for its 2 heads (column-sharded weights,
   host-transposed to [D, .] so contraction sits on partitions).
 - Attention in transposed orientation: scoresT[j,i] tiles on PE, exp on ACT
   (scale=1/8 folded in), causal masking via gpsimd affine_select on the exp
   output, PV with ones-augmented V so the softmax denominator falls out of the
   same accumulation (row 64 of the PV psum). Normalization: DVE reciprocal of
   the denom row, broadcast across 64 partitions with a K=1 fp32 matmul, DVE mul.
 - Output projection: 8-rank AllToAll reshards concat^T from head-sharded to
   token-sharded; each core then computes 512 token rows of out = concat @ Wo^T.
All matmuls bf16 with fp32 PSUM accumulation. Host pre-transposes/casts inputs.
"""

import sys

sys.path.insert(0, "/opt/trn_rl_repo")

import numpy as np
import ml_dtypes

import concourse.bass as bass
import concourse.mybir as mybir
import concourse.tile as tile
from concourse import bacc
from concourse import bass_utils

B, S, D, H = 2, 2048, 1024, 16
DK = D // H              # 64
N_CORES = 8
HPC = H // N_CORES       # heads per core (2)
EPC = HPC * DK           # 128 projected cols per core
T = B * S                # 4096 flattened tokens
TOK = T // N_CORES       # 512 output tokens per core
IT = 512                 # i (query) tile
JT = 128                 # j (key) tile
NIT = S // IT            # 4 i-tiles per batch
NJT = S // JT            # 16 j-tiles per batch
NST = T // IT            # 8 projection token tiles
ND = D // 128            # 8 contraction tiles
VST = DK + 1             # 65: V block width with ones column

bf16 = mybir.dt.bfloat16
f32 = mybir.dt.float32
f16 = mybir.dt.float16
BF = ml_dtypes.bfloat16

_CACHE: dict = {}


def _store_junk(nc, tc, out):
    import concourse.mybir as _mb
    with tc.tile_pool(name="junk", bufs=1) as jp:
        jt_ = jp.tile([128, D], _mb.dt.float32, name="junk")
        nc.vector.memset(jt_[:], 0.0)
        for tt in range(TOK // 128):
            nc.sync.dma_start(out.ap()[128 * tt:128 * (tt + 1), :], jt_[:])


def _build(mode: str, repeats: int = 1, upto: str = "full"):
    """mode: 'causal' | 'none' | 'generic'. repeats>1 builds a timing variant
    that executes the whole body N times in one NEFF. upto: 'full' | 'p2' |
    'p1' truncates after attention / projections (timing ablation only)."""
    nc = bacc.Bacc("TRN2", target_bir_lowering=False, debug=False,
                   enable_asserts=False, num_devices=N_CORES)

    xq = nc.dram_tensor("xq", [D, T], bf16, kind="ExternalInput")
    xk = nc.dram_tensor("xk", [D, T], bf16, kind="ExternalInput")
    xv = nc.dram_tensor("xv", [D, T], bf16, kind="ExternalInput")
    wq = nc.dram_tensor("wq", [D, EPC], bf16, kind="ExternalInput")
    wk = nc.dram_tensor("wk", [D, EPC], bf16, kind="ExternalInput")
    wv = nc.dram_tensor("wv", [D, EPC], bf16, kind="ExternalInput")
    wo = nc.dram_tensor("wo", [128, D], bf16, kind="ExternalInput")
    if mode == "generic":
        bias = nc.dram_tensor("bias", [S, S], bf16, kind="ExternalInput")
    out = nc.dram_tensor("out", [T, D], f16, kind="ExternalOutput")

    Exp = mybir.ActivationFunctionType.Exp
    rg = [list(range(N_CORES))]

    with tile.TileContext(nc) as tc:
      for _rep in range(repeats):
        with (
            tc.tile_pool(name="consts", bufs=1) as consts,
            tc.tile_pool(name="persist", bufs=1) as persist,
            tc.tile_pool(name="dram", bufs=1, space="DRAM") as dram,
        ):
            # --- persistent SBUF tensors ---
            wqb = consts.tile([128, ND * EPC], bf16, tag="wqb", name="wqb")
            wkb = consts.tile([128, ND * EPC], bf16, tag="wkb", name="wkb")
            wvb = consts.tile([128, ND * EPC], bf16, tag="wvb", name="wvb")
            wob = consts.tile([128, D], bf16, tag="wob", name="wob")
            wq_sb = [wqb[:, EPC * d:EPC * (d + 1)] for d in range(ND)]
            wk_sb = [wkb[:, EPC * d:EPC * (d + 1)] for d in range(ND)]
            wv_sb = [wvb[:, EPC * d:EPC * (d + 1)] for d in range(ND)]

            QT = persist.tile([128, T], bf16, tag="QT")
            KT = persist.tile([128, T], bf16, tag="KT")
            CT = persist.tile([128, T], bf16, tag="CT")
            NJ_ALL = T // JT     # 32 j-tiles across both batches
            V_all = persist.tile([128, NJ_ALL * HPC * VST], bf16, tag="V_all")
            v4 = V_all[:].rearrange("p (t h c) -> p (t h) c",
                                    t=NJ_ALL, h=HPC, c=VST)
            nc.vector.memset(v4[:, :, DK:DK + 1], 1.0)

            nc.sync.dma_start(
                wqb[:].rearrange("p (d e) -> p d e", d=ND, e=EPC),
                wq.ap().rearrange("(d p) e -> p d e", p=128))
            nc.sync.dma_start(
                wkb[:].rearrange("p (d e) -> p d e", d=ND, e=EPC),
                wk.ap().rearrange("(d p) e -> p d e", p=128))
            nc.sync.dma_start(
                wvb[:].rearrange("p (d e) -> p d e", d=ND, e=EPC),
                wv.ap().rearrange("(d p) e -> p d e", p=128))
            nc.sync.dma_start(wob[:], wo.ap())

            # ================= Phase 1: projections =================
            with (
                tc.tile_pool(name="xbig", bufs=6) as xbig,
                tc.tile_pool(name="psA", bufs=1, space="PSUM") as psA,
            ):
                for wsb, xdram, dest in ((wq_sb, xq, QT), (wk_sb, xk, KT)):
                    # d-outer, two d-tiles per DMA: each load feeds 16 st
                    # matmuls; the 8 st accumulators live in 8 PSUM banks.
                    pss = [psA.tile([128, IT], f32, tag="proj", bufs=NST,
                                    name=f"ps{st}") for st in range(NST)]
                    for d2 in range(ND // 2):
                        xt = xbig.tile([128, 2, T], bf16, tag="xbig",
                                       name="xt")
                        nc.sync.dma_start(
                            xt[:],
                            xdram.ap()[256 * d2:256 * (d2 + 1), :].rearrange(
                                "(two p) t -> p two t", p=128))
                        for half in range(2):
                            d = 2 * d2 + half
                            for st in range(NST):
                                nc.tensor.matmul(
                                    pss[st][:], wsb[d],
                                    xt[:, half, IT * st:IT * (st + 1)],
                                    start=(d == 0), stop=(d == ND - 1))
                    for st in range(NST):
                        nc.vector.tensor_copy(
                            dest[:, IT * st:IT * (st + 1)], pss[st][:])

                # V: out[j, c] for the 2 local heads; lhsT = xvT tile slices
                xvs = []
                for d2 in range(ND // 2):
                    xt = xbig.tile([128, 2, T], bf16, tag="xbig", name="xvt")
                    nc.sync.dma_start(
                        xt[:],
                        xv.ap()[256 * d2:256 * (d2 + 1), :].rearrange(
                            "(two p) t -> p two t", p=128))
                    xvs.append(xt[:, 0, :])
                    xvs.append(xt[:, 1, :])
                for jt in range(NJ_ALL):
                    psv = psA.tile([128, EPC], f32, tag="proj", bufs=NST,
                                   name="psv")
                    for d in range(ND):
                        nc.tensor.matmul(
                            psv[:], xvs[d][:, JT * jt:JT * (jt + 1)],
                            wv_sb[d], start=(d == 0), stop=(d == ND - 1))
                    dst = V_all[:, VST * HPC * jt:VST * HPC * (jt + 1)]
                    nc.vector.tensor_copy(
                        dst.rearrange("p (h c) -> p h c", h=HPC, c=VST)[:, :, 0:DK],
                        psv[:].rearrange("p (h c) -> p h c", h=HPC, c=DK))

            # ================= Phase 2: attention =================
            if upto == "p1":
                _store_junk(nc, tc, out)
                continue
            # Two heads interleaved per j-tile: breaks the pt->exp->po latency
            # chain and puts the two K=64 matmuls on different PE row groups
            # (base partitions 0/64) so they run concurrently on the array.
            with (
                tc.tile_pool(name="psP", bufs=1, space="PSUM") as psP,
                tc.tile_pool(name="psO", bufs=1, space="PSUM") as psO,
                tc.tile_pool(name="sbE", bufs=1) as sbE,
                tc.tile_pool(name="sbR", bufs=1) as sbR,
                tc.tile_pool(name="biasp", bufs=4) as biasp,
                tc.tile_pool(name="sbF", bufs=1) as sbF,
            ):
                final_q = []

                def _emit_final(fi0):
                    of = sbF.tile([128, (IT // 128) * D], f16, tag="of",
                                  bufs=2, name="of")
                    for tt in range(IT // 128):
                        t0 = fi0 + 128 * tt
                        for eh in range(2):
                            pf = psP.tile([128, IT], f32, tag="pt",
                                          bufs=5, name="pf")
                            nc.tensor.matmul(
                                pf[:], CT[:, t0:t0 + 128],
                                wob[:, 512 * eh:512 * (eh + 1)],
                                start=True, stop=True)
                            nc.vector.tensor_copy(
                                of[:, D * tt + 512 * eh:
                                   D * tt + 512 * (eh + 1)], pf[:])
                    nc.sync.dma_start(
                        out.ap()[fi0:fi0 + IT, :].rearrange(
                            "(tt p) e -> p tt e", p=128),
                        of[:].rearrange("p (tt e) -> p tt e",
                                        tt=IT // 128, e=D))

                for b in range(B):
                    tok0 = S * b
                    for it in range(NIT):
                        il0 = IT * it            # batch-local i offset
                        i0 = tok0 + il0
                        njt = (il0 + IT) // JT if mode == "causal" else NJT
                        pos = [psO.tile([VST, IT], f32, tag="po", bufs=3,
                                        name=f"po{hl}") for hl in range(HPC)]
                        pend = []   # (jl, hl, ex) exp'd tiles awaiting PV
                        for jl in range(njt):
                            jabs = NJT * b + jl
                            j0 = JT * jl             # batch-local j offset
                            diag = mode == "causal" and j0 > il0 - 1
                            # live i-columns of this block: i >= j0 (causal)
                            off = max(0, j0 - il0) if mode == "causal" else 0
                            nl = IT - off            # live width
                            bs = None
                            if mode == "generic":
                                bs = biasp.tile([128, IT], bf16, tag="bias",
                                                name="bs")
                                nc.sync.dma_start(
                                    bs[:],
                                    bias.ap()[JT * jl:JT * (jl + 1),
                                              il0:il0 + IT])
                            for hl in range(HPC):
                                pb = 64 * hl
                                pt = psP.tile([128, IT], f32, tag="pt",
                                              bufs=5, name="pt")
                                nc.tensor.matmul(
                                    pt[:, 0:nl],
                                    KT[pb:pb + DK, JT * jabs:JT * (jabs + 1)],
                                    QT[pb:pb + DK, i0 + off:i0 + IT],
                                    start=True, stop=True)
                                if bs is not None:
                                    nc.vector.tensor_add(pt[:, 0:nl],
                                                         pt[:, 0:nl],
                                                         bs[:, off:IT])
                                ex = sbE.tile([128, IT], bf16, tag="expp",
                                              bufs=12, name="ex")
                                nc.scalar.activation(ex[:, 0:nl],
                                                     pt[:, 0:nl], Exp,
                                                     scale=0.125)
                                if diag:
                                    # triangular part lives in the first JT
                                    # live cols: keep iff j0+p <= j0+f
                                    nc.gpsimd.affine_select(
                                        out=ex[:, 0:JT], in_=ex[:, 0:JT],
                                        compare_op=mybir.AluOpType.is_ge,
                                        fill=0.0,
                                        base=0,
                                        pattern=[[1, JT]],
                                        channel_multiplier=-1)
                                pend.append((jl, hl, ex, off, nl))
                            # emit PV one j-step behind so each po matmul's
                            # exp input was produced during the previous
                            # j-step's score matmuls (keeps PE from stalling)
                            while len(pend) > 2 * HPC:
                                pjl, phl, pex, poff, pnl = pend.pop(0)
                                pjabs = NJT * b + pjl
                                voff = VST * (HPC * pjabs + phl)
                                nc.tensor.matmul(pos[phl][:, poff:IT],
                                                 V_all[:, voff:voff + VST],
                                                 pex[:, 0:pnl],
                                                 start=(pjl == 0),
                                                 stop=(pjl == njt - 1))
                        for pjl, phl, pex, poff, pnl in pend:
                            pjabs = NJT * b + pjl
                            voff = VST * (HPC * pjabs + phl)
                            nc.tensor.matmul(pos[phl][:, poff:IT],
                                             V_all[:, voff:voff + VST],
                                             pex[:, 0:pnl],
                                             start=(pjl == 0),
                                             stop=(pjl == njt - 1))
                        for hl in range(HPC):
                            pb = 64 * hl
                            rec = sbR.tile([1, IT], f32, tag="rec", bufs=2,
                                           name="rec")
                            nc.vector.reciprocal(rec[:],
                                                 pos[hl][DK:DK + 1, :])
                            pbs = sbR.tile([DK, IT], f32, tag="pbs", bufs=2,
                                           name="pbs")
                            nc.gpsimd.partition_broadcast(pbs[:], rec[:])
                            nc.vector.tensor_mul(CT[pb:pb + DK, i0:i0 + IT],
                                                 pos[hl][0:DK, :], pbs[:])
                        # partial output projection for the PREVIOUS
                        # i-block (deferred one block so its division chain
                        # overlaps this block's score matmuls):
                        # out_partial[t, e] = sum_{local d} CT[d, t]*woT[d, e]
                        final_q.append(i0)
                        if len(final_q) > 1:
                            _emit_final(final_q.pop(0))

                for fi0 in final_q:
                    _emit_final(fi0)

    nc.compile()
    return nc


def _prep(inputs, mode):
    query = np.asarray(inputs["query"], np.float32)
    key = np.asarray(inputs["key"], np.float32)
    value = np.asarray(inputs["value"], np.float32)
    Wq = np.asarray(inputs["Wq"], np.float32)
    Wk = np.asarray(inputs["Wk"], np.float32)
    Wv = np.asarray(inputs["Wv"], np.float32)
    Wo = np.asarray(inputs["Wo"], np.float32)

    xqT = np.ascontiguousarray(query.reshape(T, D).T).astype(BF)
    xkT = np.ascontiguousarray(key.reshape(T, D).T).astype(BF)
    xvT = np.ascontiguousarray(value.reshape(T, D).T).astype(BF)
    woT = np.ascontiguousarray(Wo.T).astype(BF)
    woT_loc = [np.ascontiguousarray(woT[128 * c:128 * (c + 1), :])
               for c in range(N_CORES)]
    wqT = [np.ascontiguousarray(Wq[EPC * c:EPC * (c + 1), :].T).astype(BF)
           for c in range(N_CORES)]
    wkT = [np.ascontiguousarray(Wk[EPC * c:EPC * (c + 1), :].T).astype(BF)
           for c in range(N_CORES)]
    wvT = [np.ascontiguousarray(Wv[EPC * c:EPC * (c + 1), :].T).astype(BF)
           for c in range(N_CORES)]

    biasT = None
    if mode == "generic":
        m2 = np.asarray(inputs["mask"])[0, 0]
        biasT = np.ascontiguousarray(
            np.where(m2.T == 0, np.float32(-1e9), np.float32(0.0))).astype(BF)

    in_maps = []
    for c in range(N_CORES):
        m = {"xq": xqT, "xk": xkT, "xv": xvT,
             "wq": wqT[c], "wk": wkT[c], "wv": wvT[c], "wo": woT_loc[c]}
        if biasT is not None:
            m["bias"] = biasT
        in_maps.append(m)
    return in_maps


def _mask_mode(mask):
    m2 = np.asarray(mask)[0, 0]
    if (m2 == 1).all():
        return "none"
    if np.array_equal(m2 != 0, np.tril(np.ones(m2.shape, dtype=bool))):
        return "causal"
    return "generic"


def kernel(**inputs) -> np.ndarray:
    mode = _mask_mode(inputs["mask"])
    if mode not in _CACHE:
        _CACHE[mode] = _build(mode)
    nc = _CACHE[mode]
    in_maps = _prep(inputs, mode)
    res = bass_utils.run_bass_kernel_spmd(nc, in_maps,
                                          core_ids=list(range(N_CORES)))
    out = res.results[0]["out"].astype(np.float32)
    for c in range(1, N_CORES):
        out += res.results[c]["out"]
    return out.reshape(B, S, D)


if __name__ == "__main__":
    rng = np.random.default_rng(0)
    inputs = {
        "query": rng.standard_normal((B, S, D)).astype(np.float32),
        "key": rng.standard_normal((B, S, D)).astype(np.float32),
        "value": rng.standard_normal((B, S, D)).astype(np.float32),
        "mask": np.tril(np.ones((S, S), np.int32))[None, None],
        "Wq": (rng.standard_normal((D, D)) / 32).astype(np.float32),
        "Wk": (rng.standard_normal((D, D)) / 32).astype(np.float32),
        "Wv": (rng.standard_normal((D, D)) / 32).astype(np.float32),
        "Wo": (rng.standard_normal((D, D)) / 32).astype(np.float32),
    }
    got = kernel(**inputs)
    print("kernel ran, out shape", got.shape, "finite:", np.isfinite(got).all())
